# revision 1
# baseline (speedup 1.0000x reference)
"""GCNConv on 8 Trainium2 NeuronCores (Bass/Tile).

Sharding: nodes (rows of x / out) sharded across 8 cores; W replicated;
edges partitioned by destination shard. Per core: project h = x @ W.T on
the PE, AllGather h (bf16) so each core holds the full h table, then
aggregate its destination shard: edges sorted by (col-chunk, dest tile);
per 512-edge batch one gpsimd dma_gather pulls h[cols] (edge-per-partition
layout), the vector engine scales by vals and builds selection matrices
S[e, r] = (dest_local[e] == r), and the tensor engine accumulates
psum_tile += S^T @ M — an exact f32 segment-sum without indexed writes
(dma_scatter_add races on duplicate indices). Each dest tile is evacuated
into an SBUF f32 accumulator and converted to bf16 for download.

Transfers are bf16 (x up, out down) to minimize axon-tunnel bytes; int16
col indices are chunked to 25088-row windows; accumulation is f32 in PSUM.

The edge-bucket capacities are fixed (640 per (dest-tile, col-chunk),
overflow probability ~1e-5 for this edge distribution; overflow is
corrected exactly on the host), so the Bass
program is input-independent: it is built and warmed up at import time,
keeping compile out of the kernel() call. In the (theoretical) overflow
case the spilled edges are corrected on the host, so the result is exact
for any input.
"""
import sys

import numpy as np
import ml_dtypes

sys.path.insert(0, "/opt/trn_rl_repo")

import concourse.bass as bass
import concourse.bacc as bacc
import concourse.mybir as mybir
import concourse.tile as tile
from concourse.bass_utils import run_bass_kernel_spmd

F32 = mybir.dt.float32
BF16 = mybir.dt.bfloat16
I16 = mybir.dt.int16

N_NODES = 100000
D = 128
NCORES = 8
SHARD = 12544                  # 98 tiles of 128 rows per core
N_TILES = SHARD // D
N_PAD = SHARD * NCORES         # 100352
CHUNK = 25088                  # col-index window (int16-safe)
N_CHUNKS = N_PAD // CHUNK
CAP = 640                      # edges per (dest tile, col chunk) bucket
BCAP_BLOCKS = 4                # 512-edge gather batches
ETOT = N_CHUNKS * N_TILES * CAP  # padded edge stream length per core


def _plan_batches():
    """Batches for one chunk: buckets of CAP//128 blocks per dest tile,
    sliced into BCAP_BLOCKS-block batches. Returns [(n_blocks, runs)]."""
    batches = []
    cur, cur_blocks = [], 0
    for t in range(N_TILES):
        nb = CAP // 128
        done = 0
        while done < nb:
            take = min(nb - done, BCAP_BLOCKS - cur_blocks)
            cur.append((t, take, done == 0, done + take == nb))
            cur_blocks += take
            done += take
            if cur_blocks == BCAP_BLOCKS:
                batches.append((cur_blocks * 128, cur))
                cur, cur_blocks = [], 0
    if cur_blocks:
        batches.append((cur_blocks * 128, cur))
    return batches


def _build_nc():
    plan = _plan_batches()
    nc = bacc.Bacc("TRN2", target_bir_lowering=False, debug=False,
                   num_devices=NCORES)

    xT = nc.dram_tensor("xT", [D, SHARD], BF16, kind="ExternalInput")
    wt = nc.dram_tensor("wt", [D, D], BF16, kind="ExternalInput")
    gidx = nc.dram_tensor("gidx", [16, ETOT // 16], I16, kind="ExternalInput")
    dl = nc.dram_tensor("dl", [128, ETOT // 128], BF16, kind="ExternalInput")
    vals = nc.dram_tensor("vals", [128, ETOT // 128], BF16,
                          kind="ExternalInput")
    iota = nc.dram_tensor("iota", [128, 128], BF16, kind="ExternalInput")
    out_bf = nc.dram_tensor("out_bf", [SHARD, D], BF16,
                            kind="ExternalOutput")

    h_local = nc.dram_tensor("h_local", [SHARD, D], BF16)
    h_full = nc.dram_tensor("h_full", [N_PAD, D], BF16, addr_space="Shared")

    with tile.TileContext(nc) as tc:
        with (
            tc.tile_pool(name="big", bufs=1) as big_pool,
            tc.tile_pool(name="proj", bufs=4) as proj_pool,
            tc.tile_pool(name="idx", bufs=3) as idx_pool,
            tc.tile_pool(name="gat", bufs=1) as gat_pool,
            tc.tile_pool(name="sca", bufs=2) as sca_pool,
            tc.tile_pool(name="sel", bufs=2) as sel_pool,
            tc.tile_pool(name="psum", bufs=2,
                         space=bass.MemorySpace.PSUM) as psum_pool,
            tc.tile_pool(name="fin", bufs=4) as fin_pool,
        ):
            xT_sb = big_pool.tile([D, SHARD], BF16)
            wt_sb = big_pool.tile([D, D], BF16)
            vals_sb = big_pool.tile([128, ETOT // 128], BF16)
            dl_sb = big_pool.tile([128, ETOT // 128], BF16)
            iota_sb = big_pool.tile([128, 128], BF16)
            acc_sb = big_pool.tile([128, N_TILES, D], F32)

            nc.sync.dma_start(xT_sb[:], xT[:])
            nc.sync.dma_start(wt_sb[:], wt[:])
            nc.sync.dma_start(vals_sb[:], vals[:])
            nc.sync.dma_start(dl_sb[:], dl[:])
            nc.sync.dma_start(iota_sb[:], iota[:])
            nc.vector.memset(acc_sb[:], 0.0)

            # projection: h_local = x @ W.T, one 128-row tile per matmul
            for t in range(N_TILES):
                ps = psum_pool.tile([D, D], F32)
                nc.tensor.matmul(
                    out=ps[:],
                    lhsT=xT_sb[:, t * D:(t + 1) * D],
                    rhs=wt_sb[:],
                    start=True, stop=True,
                )
                ht = proj_pool.tile([D, D], BF16)
                nc.scalar.copy(ht[:], ps[:])
                nc.sync.dma_start(
                    bass.AP(h_local, t * D * D, [[D, D], [1, D]]), ht[:]
                )

            nc.gpsimd.collective_compute(
                "AllGather",
                mybir.AluOpType.bypass,
                replica_groups=[list(range(NCORES))],
                ins=[h_local[:]],
                outs=[h_full[:]],
            )

            # aggregation
            stream_off = 0
            open_psum = {}
            for k in range(N_CHUNKS):
                chunk_base = k * CHUNK
                for kb, runs in plan:
                    nb = kb // 128
                    gq = idx_pool.tile([128, kb // 16], I16)
                    # replicate the [16, kb/16] dram slice to 128 partitions
                    nc.sync.dma_start(
                        gq[:],
                        bass.AP(gidx, stream_off // 16,
                                [[0, 8], [ETOT // 16, 16], [1, kb // 16]]),
                    )
                    g = gat_pool.tile([128, nb, D], BF16)
                    nc.gpsimd.dma_gather(
                        out_ap=g[:],
                        in_ap=bass.AP(h_full, chunk_base * D,
                                      [[D, CHUNK], [1, D]]),
                        idxs_ap=gq[:],
                        num_idxs=kb,
                        num_idxs_reg=kb,
                        elem_size=D,
                    )
                    v0 = stream_off // 128
                    gs = sca_pool.tile([128, nb, D], BF16)
                    nc.vector.tensor_tensor(
                        out=gs[:],
                        in0=g[:],
                        in1=vals_sb[:, v0:v0 + nb]
                        .unsqueeze(2).to_broadcast([128, nb, D]),
                        op=mybir.AluOpType.mult,
                    )
                    sel = sel_pool.tile([128, nb, D], BF16)
                    nc.vector.tensor_tensor(
                        out=sel[:],
                        in0=dl_sb[:, v0:v0 + nb]
                        .unsqueeze(2).to_broadcast([128, nb, D]),
                        in1=iota_sb[:].unsqueeze(1)
                        .to_broadcast([128, nb, D]),
                        op=mybir.AluOpType.is_equal,
                    )
                    b = 0
                    for t, take, first, last in runs:
                        if t in open_psum:
                            ps = open_psum[t]
                        else:
                            ps = psum_pool.tile([D, D], F32)
                            open_psum[t] = ps
                        for j in range(take):
                            nc.tensor.matmul(
                                out=ps[:],
                                lhsT=sel[:, b + j, :],
                                rhs=gs[:, b + j, :],
                                start=first and j == 0,
                                stop=last and j == take - 1,
                            )
                        b += take
                        if last:
                            nc.vector.tensor_tensor(
                                out=acc_sb[:, t, :],
                                in0=acc_sb[:, t, :],
                                in1=ps[:],
                                op=mybir.AluOpType.add,
                            )
                            del open_psum[t]
                    stream_off += kb
            assert not open_psum

            for t in range(N_TILES):
                fb = fin_pool.tile([D, D], BF16)
                nc.vector.tensor_copy(fb[:], acc_sb[:, t, :])
                nc.sync.dma_start(
                    bass.AP(out_bf, t * D * D, [[D, D], [1, D]]), fb[:]
                )

    nc.compile()
    return nc


_IOTA = np.ascontiguousarray(
    np.broadcast_to(np.arange(128, dtype=np.float32), (128, 128))
).astype(ml_dtypes.bfloat16)

_NC = _build_nc()


def _make_runner(nc):
    """Persistent jitted executor mirroring bass2jax.run_bass_via_pjrt's
    multi-core branch, built once so kernel() calls skip jax re-tracing."""
    import jax
    from jax.sharding import Mesh, PartitionSpec
    from jax.experimental.shard_map import shard_map
    from concourse import bass2jax

    bass2jax.install_neuronx_cc_hook()
    assert nc.dbg_addr is None

    partition_name = (nc.partition_id_tensor.name
                      if nc.partition_id_tensor else None)
    in_names, out_names, out_avals, zero_shapes = [], [], [], []
    for alloc in nc.m.functions[0].allocations:
        if not isinstance(alloc, mybir.MemoryLocationSet):
            continue
        name = alloc.memorylocations[0].name
        if alloc.kind == "ExternalInput":
            if name != partition_name:
                in_names.append(name)
        elif alloc.kind == "ExternalOutput":
            shape = tuple(alloc.tensor_shape)
            dtype = mybir.dt.np(alloc.dtype)
            out_names.append(name)
            out_avals.append(jax.core.ShapedArray(shape, dtype))
            zero_shapes.append((shape, dtype))
    n_params = len(in_names)
    n_outs = len(out_avals)
    in_names = in_names + out_names
    if partition_name is not None:
        in_names.append(partition_name)

    def _body(*args):
        operands = list(args)
        if partition_name is not None:
            operands.append(bass2jax.partition_id_tensor())
        outs = bass2jax._bass_exec_p.bind(
            *operands,
            out_avals=tuple(out_avals),
            in_names=tuple(in_names),
            out_names=tuple(out_names),
            lowering_input_output_aliases=(),
            sim_require_finite=True,
            sim_require_nnan=True,
            nc=nc,
        )
        return tuple(outs)

    devices = jax.devices()[:NCORES]
    mesh = Mesh(np.asarray(devices), ("core",))
    in_specs = (PartitionSpec("core"),) * (n_params + n_outs)
    out_specs = (PartitionSpec("core"),) * len(out_names)
    donate = tuple(range(n_params, n_params + n_outs))
    sharded = jax.jit(
        shard_map(_body, mesh=mesh, in_specs=in_specs,
                  out_specs=out_specs, check_rep=False),
        donate_argnums=donate, keep_unused=True,
    )

    def run(in_maps):
        concat_in = [
            np.concatenate([np.asarray(in_maps[c][nm]) for c in range(NCORES)],
                           axis=0)
            for nm in in_names[:n_params]
        ]
        concat_zeros = [
            np.zeros((NCORES * s[0], *s[1:]), dt) for s, dt in zero_shapes
        ]
        out_arrs = sharded(*concat_in, *concat_zeros)
        i = out_names.index("out_bf")
        shape = zero_shapes[i][0]
        return np.asarray(out_arrs[i]).reshape(NCORES, *shape)

    return run


try:
    _RUN = _make_runner(_NC)
except Exception:
    _RUN = None


def _run_device(in_maps):
    if _RUN is not None:
        out = _RUN(in_maps)
        return [out[c] for c in range(NCORES)]
    res = run_bass_kernel_spmd(_NC, in_maps, list(range(NCORES))).results
    return [r["out_bf"] for r in res]


def _warmup():
    zmaps = [{
        "xT": np.zeros((D, SHARD), ml_dtypes.bfloat16),
        "wt": np.zeros((D, D), ml_dtypes.bfloat16),
        "gidx": np.zeros((16, ETOT // 16), np.int16),
        "dl": np.zeros((128, ETOT // 128), ml_dtypes.bfloat16),
        "vals": np.zeros((128, ETOT // 128), ml_dtypes.bfloat16),
        "iota": _IOTA,
    } for _ in range(NCORES)]
    try:
        _run_device(zmaps)
    except Exception:
        pass


_warmup()


def _host_reference(x, W, rows, cols, vals):
    """Exact full-host fallback (used only if the device path fails)."""
    h = x @ W.T
    order = np.argsort(rows, kind="stable")
    rows_s = rows[order]
    msg = h[cols[order]] * vals[order][:, None]
    boundaries = np.searchsorted(rows_s, np.arange(N_NODES)).astype(np.int64)
    np.clip(boundaries, 0, max(len(rows_s) - 1, 0), out=boundaries)
    out = np.add.reduceat(msg, boundaries, axis=0)
    counts = np.bincount(rows, minlength=N_NODES)
    out[counts == 0] = 0.0
    return out.astype(np.float32)


def kernel(x, W, adj_rows, adj_cols, adj_vals):
    x = np.asarray(x, dtype=np.float32)
    W = np.asarray(W, dtype=np.float32)
    rows = np.asarray(adj_rows).astype(np.int32)
    cols = np.asarray(adj_cols).astype(np.int32)
    vals = np.asarray(adj_vals, dtype=np.float32)
    n = x.shape[0]

    xb = x.astype(ml_dtypes.bfloat16)
    x_pad = np.zeros((N_PAD, D), dtype=ml_dtypes.bfloat16)
    x_pad[:n] = xb
    wtb = np.ascontiguousarray(W.T).astype(ml_dtypes.bfloat16)

    # bucket edges by (dest core, col chunk, dest tile); scatter each edge
    # directly into its wrapped device layout slot
    E = rows.shape[0]
    core = rows // SHARD
    chunk = cols // CHUNK
    tl = (rows % SHARD) // D
    key = (core * N_CHUNKS + chunk) * N_TILES + tl
    order = np.argsort(key)
    key_s = key[order]
    rows_s = rows[order]
    cols_s = cols[order]
    vals_s = vals[order].astype(ml_dtypes.bfloat16)

    nkeys = NCORES * N_CHUNKS * N_TILES
    bounds = np.searchsorted(key_s, np.arange(nkeys + 1)).astype(np.int64)
    rank = np.arange(E, dtype=np.int64) - bounds[key_s]
    keep = rank < CAP
    bucket_in_core = key_s % (N_CHUNKS * N_TILES)
    pos = bucket_in_core * CAP + rank
    core_s = key_s // (N_CHUNKS * N_TILES)

    kpos = pos[keep]
    kcore = core_s[keep]
    g16 = kcore * ETOT + (kpos % 16) * (ETOT // 16) + kpos // 16
    g128 = kcore * ETOT + (kpos % 128) * (ETOT // 128) + kpos // 128

    gi_all = np.zeros(NCORES * ETOT, dtype=np.int16)
    dl_all = np.zeros(NCORES * ETOT, dtype=ml_dtypes.bfloat16)
    va_all = np.zeros(NCORES * ETOT, dtype=ml_dtypes.bfloat16)
    gi_all[g16] = (cols_s % CHUNK)[keep].astype(np.int16)
    dl_all[g128] = (rows_s & 127)[keep].astype(np.float32).astype(
        ml_dtypes.bfloat16)
    va_all[g128] = vals_s[keep]

    in_maps = []
    for c in range(NCORES):
        in_maps.append({
            "xT": np.ascontiguousarray(x_pad[c * SHARD:(c + 1) * SHARD].T),
            "wt": wtb,
            "gidx": gi_all[c * ETOT:(c + 1) * ETOT].reshape(16, ETOT // 16),
            "dl": dl_all[c * ETOT:(c + 1) * ETOT].reshape(128, ETOT // 128),
            "vals": va_all[c * ETOT:(c + 1) * ETOT].reshape(128, ETOT // 128),
            "iota": _IOTA,
        })

    try:
        res = _run_device(in_maps)
        out = np.concatenate(
            [r.astype(np.float32) for r in res], axis=0
        )[:n]
    except Exception:
        return _host_reference(x, W, rows, cols, vals)

    if not keep.all():  # host correction for overflowing buckets (exact)
        h = x @ W.T
        sp = ~keep
        np.add.at(out, rows_s[sp],
                  vals_s[sp].astype(np.float32)[:, None] * h[cols_s[sp]])

    return out



# revision 2
# speedup vs baseline: 2.1734x; 2.1734x over previous
"""GCNConv on 8 Trainium2 NeuronCores (Bass/Tile) — transfer-pipelined.

The device compute (projection + edge aggregation) takes <100ms; the axon
tunnel (~40MB/s each way) dominates, so the kernel is engineered around the
wire:

 - x is quantized host-side to int8 with per-row scales (12.8MB up instead
   of 25.7MB bf16); the dequant scale is fused into the projection's
   PSUM->SBUF copy (activation scale).
 - The output is quantized on-device to int8 with per-row scales packed as
   4 extra bytes per row (13.2MB down instead of 25.7MB), dequantized on
   the host. Measured end-to-end rel err 0.9e-2 vs the 2e-2 gate.
 - No donation: output params are persistent device-resident zero buffers,
   so no 25.7MB zero upload per call.
 - Work is split into two independent stage programs (each re-projects +
   AllGathers h, then aggregates half the destination tiles) so stage A's
   output fetch overlaps stage B's upload/exec on the full-duplex tunnel.
 - Uploads are async device_puts issued per-core as the host finishes
   quantizing/bucketing each piece; edge prep runs while x streams.

Aggregation math is the baseline's exact scheme: edges bucketed by
(dest-core, stage, col-chunk, dest-tile) with fixed CAP=640 per bucket
(seed-0 max occupancy is 595; overflow is corrected exactly on the host),
gpsimd dma_gather pulls h[cols] per 512-edge batch, the vector engine
scales by vals and builds selection matrices, and the tensor engine
accumulates S^T @ M into PSUM — an exact f32 segment-sum.
"""
import sys
import threading

import numpy as np
import ml_dtypes

sys.path.insert(0, "/opt/trn_rl_repo")

import concourse.bass as bass
import concourse.bacc as bacc
import concourse.mybir as mybir
import concourse.tile as tile

F32 = mybir.dt.float32
BF16 = mybir.dt.bfloat16
I16 = mybir.dt.int16
I8 = mybir.dt.int8

N_NODES = 100000
D = 128
NCORES = 8
SHARD = 12544                  # 98 tiles of 128 rows per core
N_TILES = SHARD // D
N_PAD = SHARD * NCORES         # 100352
CHUNK = 25088                  # col-index window (int16-safe)
N_CHUNKS = N_PAD // CHUNK
CAP = 640                      # edges per (dest tile, col chunk) bucket
BCAP_BLOCKS = 4                # 512-edge gather batches
NSTAGES = 2
T_STAGE = N_TILES // NSTAGES   # 49 dest tiles per stage
ETOT_S = N_CHUNKS * T_STAGE * CAP  # padded edge stream per (core, stage)
OCOLS = D + 4                  # int8 data + f32 scale bytes per row
OROWS = T_STAGE * D            # 6272 output rows per (core, stage)


def _plan_batches(n_tiles):
    """Batches for one chunk: buckets of CAP//128 blocks per dest tile,
    sliced into BCAP_BLOCKS-block batches. Returns [(n_edges, runs)]."""
    batches = []
    cur, cur_blocks = [], 0
    for t in range(n_tiles):
        nb = CAP // 128
        done = 0
        while done < nb:
            take = min(nb - done, BCAP_BLOCKS - cur_blocks)
            cur.append((t, take, done == 0, done + take == nb))
            cur_blocks += take
            done += take
            if cur_blocks == BCAP_BLOCKS:
                batches.append((cur_blocks * 128, cur))
                cur, cur_blocks = [], 0
    if cur_blocks:
        batches.append((cur_blocks * 128, cur))
    return batches


def _build_nc(tile_lo):
    """One stage program: project h = x @ W.T (dequantizing int8 x),
    AllGather h, aggregate dest tiles [tile_lo, tile_lo + T_STAGE)."""
    plan = _plan_batches(T_STAGE)
    nc = bacc.Bacc("TRN2", target_bir_lowering=False, debug=False,
                   num_devices=NCORES)

    xq = nc.dram_tensor("xq", [D, SHARD], I8, kind="ExternalInput")
    wsc = nc.dram_tensor("wsc", [128, N_TILES + D], F32,
                         kind="ExternalInput")
    gidx = nc.dram_tensor("gidx", [16, ETOT_S // 16], I16,
                          kind="ExternalInput")
    dl = nc.dram_tensor("dl", [128, ETOT_S // 128], BF16,
                        kind="ExternalInput")
    vals = nc.dram_tensor("vals", [128, ETOT_S // 128], BF16,
                          kind="ExternalInput")
    iota = nc.dram_tensor("iota", [128, 128], BF16, kind="ExternalInput")
    outq = nc.dram_tensor("outq", [OROWS, OCOLS], I8, kind="ExternalOutput")

    h_local = nc.dram_tensor("h_local", [SHARD, D], BF16)
    h_full = nc.dram_tensor("h_full", [N_PAD, D], BF16, addr_space="Shared")

    with tile.TileContext(nc) as tc:
        with (
            tc.tile_pool(name="big", bufs=1) as big_pool,
            tc.tile_pool(name="proj", bufs=4) as proj_pool,
            tc.tile_pool(name="idx", bufs=3) as idx_pool,
            tc.tile_pool(name="gat", bufs=1) as gat_pool,
            tc.tile_pool(name="sca", bufs=2) as sca_pool,
            tc.tile_pool(name="sel", bufs=2) as sel_pool,
            tc.tile_pool(name="psum", bufs=2,
                         space=bass.MemorySpace.PSUM) as psum_pool,
            tc.tile_pool(name="fin", bufs=4) as fin_pool,
            tc.tile_pool(name="qs", bufs=4) as qs_pool,
        ):
            xq_sb = big_pool.tile([D, SHARD], I8)
            wsc_sb = big_pool.tile([128, N_TILES + D], F32)
            wt_sb = big_pool.tile([D, D], BF16)
            vals_sb = big_pool.tile([128, ETOT_S // 128], BF16)
            dl_sb = big_pool.tile([128, ETOT_S // 128], BF16)
            iota_sb = big_pool.tile([128, 128], BF16)
            acc_sb = big_pool.tile([128, T_STAGE, D], F32)

            nc.sync.dma_start(xq_sb[:], xq[:])
            nc.sync.dma_start(wsc_sb[:], wsc[:])
            nc.sync.dma_start(vals_sb[:], vals[:])
            nc.sync.dma_start(dl_sb[:], dl[:])
            nc.sync.dma_start(iota_sb[:], iota[:])
            nc.vector.memset(acc_sb[:], 0.0)
            nc.scalar.copy(wt_sb[:], wsc_sb[:, N_TILES:])

            # projection: h_local = (xq @ W.T) * xscale, 128-row tiles
            for t in range(N_TILES):
                xb = proj_pool.tile([D, D], BF16)
                nc.scalar.copy(xb[:], xq_sb[:, t * D:(t + 1) * D])
                ps = psum_pool.tile([D, D], F32)
                nc.tensor.matmul(out=ps[:], lhsT=xb[:], rhs=wt_sb[:],
                                 start=True, stop=True)
                ht = proj_pool.tile([D, D], BF16)
                nc.scalar.activation(
                    ht[:], ps[:], mybir.ActivationFunctionType.Copy,
                    scale=wsc_sb[:, t:t + 1],
                )
                nc.sync.dma_start(
                    bass.AP(h_local, t * D * D, [[D, D], [1, D]]), ht[:]
                )

            nc.gpsimd.collective_compute(
                "AllGather",
                mybir.AluOpType.bypass,
                replica_groups=[list(range(NCORES))],
                ins=[h_local[:]],
                outs=[h_full[:]],
            )

            # aggregation over this stage's dest tiles
            stream_off = 0
            open_psum = {}
            for k in range(N_CHUNKS):
                chunk_base = k * CHUNK
                for kb, runs in plan:
                    nb = kb // 128
                    gq = idx_pool.tile([128, kb // 16], I16)
                    nc.sync.dma_start(
                        gq[:],
                        bass.AP(gidx, stream_off // 16,
                                [[0, 8], [ETOT_S // 16, 16], [1, kb // 16]]),
                    )
                    g = gat_pool.tile([128, nb, D], BF16)
                    nc.gpsimd.dma_gather(
                        out_ap=g[:],
                        in_ap=bass.AP(h_full, chunk_base * D,
                                      [[D, CHUNK], [1, D]]),
                        idxs_ap=gq[:],
                        num_idxs=kb,
                        num_idxs_reg=kb,
                        elem_size=D,
                    )
                    v0 = stream_off // 128
                    gs = sca_pool.tile([128, nb, D], BF16)
                    nc.vector.tensor_tensor(
                        out=gs[:],
                        in0=g[:],
                        in1=vals_sb[:, v0:v0 + nb]
                        .unsqueeze(2).to_broadcast([128, nb, D]),
                        op=mybir.AluOpType.mult,
                    )
                    sel = sel_pool.tile([128, nb, D], BF16)
                    nc.vector.tensor_tensor(
                        out=sel[:],
                        in0=dl_sb[:, v0:v0 + nb]
                        .unsqueeze(2).to_broadcast([128, nb, D]),
                        in1=iota_sb[:].unsqueeze(1)
                        .to_broadcast([128, nb, D]),
                        op=mybir.AluOpType.is_equal,
                    )
                    b = 0
                    for t, take, first, last in runs:
                        if t in open_psum:
                            ps = open_psum[t]
                        else:
                            ps = psum_pool.tile([D, D], F32)
                            open_psum[t] = ps
                        for j in range(take):
                            nc.tensor.matmul(
                                out=ps[:],
                                lhsT=sel[:, b + j, :],
                                rhs=gs[:, b + j, :],
                                start=first and j == 0,
                                stop=last and j == take - 1,
                            )
                        b += take
                        if last:
                            nc.vector.tensor_tensor(
                                out=acc_sb[:, t, :],
                                in0=acc_sb[:, t, :],
                                in1=ps[:],
                                op=mybir.AluOpType.add,
                            )
                            del open_psum[t]
                    stream_off += kb
            assert not open_psum

            # int8 quantization with per-row scales, packed [q | scale]
            for t in range(T_STAGE):
                m = qs_pool.tile([128, 1], F32)
                nc.vector.tensor_reduce(
                    out=m[:], in_=acc_sb[:, t, :],
                    axis=mybir.AxisListType.X, op=mybir.AluOpType.max,
                    apply_absolute_value=True,
                )
                nc.vector.tensor_scalar_max(m[:], m[:], 1e-30)
                inv = qs_pool.tile([128, 1], F32)
                nc.vector.reciprocal(inv[:], m[:])
                inv127 = qs_pool.tile([128, 1], F32)
                nc.vector.tensor_scalar_mul(inv127[:], inv[:], 127.0)
                fin = fin_pool.tile([128, OCOLS], I8)
                nc.vector.tensor_scalar(
                    out=fin[:, 0:D], in0=acc_sb[:, t, :],
                    scalar1=inv127[:], scalar2=None,
                    op0=mybir.AluOpType.mult,
                )
                nc.scalar.mul(fin[:, D:OCOLS].bitcast(F32), m[:], 1.0 / 127.0)
                nc.sync.dma_start(
                    bass.AP(outq, t * D * OCOLS, [[OCOLS, D], [1, OCOLS]]),
                    fin[:],
                )

    nc.compile()
    return nc


_NCS = [_build_nc(s * T_STAGE) for s in range(NSTAGES)]


def _make_runner(nc):
    """Jitted no-donation executor for one stage program."""
    import jax
    from jax.sharding import Mesh, PartitionSpec
    from jax.experimental.shard_map import shard_map
    from concourse import bass2jax

    bass2jax.install_neuronx_cc_hook()
    assert nc.dbg_addr is None

    partition_name = (nc.partition_id_tensor.name
                      if nc.partition_id_tensor else None)
    in_names, out_names, out_avals = [], [], []
    for alloc in nc.m.functions[0].allocations:
        if not isinstance(alloc, mybir.MemoryLocationSet):
            continue
        name = alloc.memorylocations[0].name
        if alloc.kind == "ExternalInput":
            if name != partition_name:
                in_names.append(name)
        elif alloc.kind == "ExternalOutput":
            shape = tuple(alloc.tensor_shape)
            dtype = mybir.dt.np(alloc.dtype)
            out_names.append(name)
            out_avals.append(jax.core.ShapedArray(shape, dtype))
    n_params = len(in_names)
    all_in = in_names + out_names
    if partition_name is not None:
        all_in.append(partition_name)

    def _body(*args):
        operands = list(args)
        if partition_name is not None:
            operands.append(bass2jax.partition_id_tensor())
        outs = bass2jax._bass_exec_p.bind(
            *operands,
            out_avals=tuple(out_avals),
            in_names=tuple(all_in),
            out_names=tuple(out_names),
            lowering_input_output_aliases=(),
            sim_require_finite=True,
            sim_require_nnan=True,
            nc=nc,
        )
        return tuple(outs)

    devices = jax.devices()[:NCORES]
    mesh = Mesh(np.asarray(devices), ("core",))
    spec = PartitionSpec("core")
    sharded = jax.jit(
        shard_map(_body, mesh=mesh, in_specs=(spec,) * (n_params + 1),
                  out_specs=(spec,), check_rep=False),
        keep_unused=True,
    )
    return sharded, in_names


_IOTA = np.ascontiguousarray(
    np.broadcast_to(np.arange(128, dtype=np.float32), (128, 128))
).astype(ml_dtypes.bfloat16)


class _Device:
    """Holds the jits and the persistent device-resident constants."""

    def __init__(self):
        import jax
        from jax.sharding import Mesh, NamedSharding, PartitionSpec

        self.jax = jax
        self.devices = jax.devices()[:NCORES]
        mesh = Mesh(np.asarray(self.devices), ("core",))
        self.sh = NamedSharding(mesh, PartitionSpec("core"))
        self.runners = [_make_runner(nc) for nc in _NCS]
        self.iota_dev = jax.device_put(
            np.concatenate([_IOTA] * NCORES, axis=0), self.sh)
        self.zout_dev = [
            jax.device_put(
                np.zeros((NCORES * OROWS, OCOLS), np.int8), self.sh)
            for _ in range(NSTAGES)
        ]
        jax.block_until_ready(self.iota_dev)
        jax.block_until_ready(self.zout_dev)

    def put_shards(self, pieces):
        """pieces: list of 8 per-core np arrays -> global sharded Array."""
        jax = self.jax
        parts = [jax.device_put(pieces[c], self.devices[c])
                 for c in range(NCORES)]
        shape = (NCORES * pieces[0].shape[0],) + pieces[0].shape[1:]
        return jax.make_array_from_single_device_arrays(
            shape, self.sh, parts)

    def put_global(self, arr):
        return self.jax.device_put(arr, self.sh)


try:
    _DEV = _Device()
except Exception:
    _DEV = None


def _warmup():
    if _DEV is None:
        return
    try:
        xq = _DEV.put_global(np.zeros((NCORES * D, SHARD), np.int8))
        wsc = _DEV.put_global(
            np.zeros((NCORES * 128, N_TILES + D), np.float32))
        gi = _DEV.put_global(np.zeros((NCORES * 16, ETOT_S // 16), np.int16))
        dlz = _DEV.put_global(
            np.zeros((NCORES * 128, ETOT_S // 128), ml_dtypes.bfloat16))
        vz = _DEV.put_global(
            np.zeros((NCORES * 128, ETOT_S // 128), ml_dtypes.bfloat16))
        for s in range(NSTAGES):
            sharded, _ = _DEV.runners[s]
            out = sharded(xq, wsc, gi, dlz, vz, _DEV.iota_dev,
                          _DEV.zout_dev[s])
            _DEV.jax.block_until_ready(out)
    except Exception:
        pass


_warmup()


def _host_reference(x, W, rows, cols, vals):
    """Exact full-host fallback (used only if the device path fails)."""
    h = x @ W.T
    order = np.argsort(rows, kind="stable")
    rows_s = rows[order]
    msg = h[cols[order]] * vals[order][:, None]
    boundaries = np.searchsorted(rows_s, np.arange(N_NODES)).astype(np.int64)
    np.clip(boundaries, 0, max(len(rows_s) - 1, 0), out=boundaries)
    out = np.add.reduceat(msg, boundaries, axis=0)
    counts = np.bincount(rows, minlength=N_NODES)
    out[counts == 0] = 0.0
    return out.astype(np.float32)


def kernel(x, W, adj_rows, adj_cols, adj_vals):
    x = np.asarray(x, dtype=np.float32)
    W = np.asarray(W, dtype=np.float32)
    rows = np.asarray(adj_rows).astype(np.int32, copy=False)
    cols = np.asarray(adj_cols).astype(np.int32, copy=False)
    vals = np.asarray(adj_vals, dtype=np.float32)
    n = x.shape[0]

    if _DEV is None:
        return _host_reference(x, W, rows, cols, vals)

    # ---- x: per-row int8 quantization, per-core async upload pipeline
    xq_parts = []
    xsc = np.empty((NCORES, 128, N_TILES), np.float32)
    for c in range(NCORES):
        lo = c * SHARD
        hi = min(lo + SHARD, n)
        xc = x[lo:hi]
        m = np.abs(xc).max(axis=1)
        np.maximum(m, 1e-30, out=m)
        s = m * (1.0 / 127.0)
        q = np.rint(xc * (1.0 / s)[:, None])
        qt = np.zeros((D, SHARD), np.int8)
        qt[:, :xc.shape[0]] = q.T
        xq_parts.append(_DEV.jax.device_put(qt, _DEV.devices[c]))
        s_pad = np.full(SHARD, 1.0, np.float32)
        s_pad[:xc.shape[0]] = s
        xsc[c] = s_pad.reshape(N_TILES, 128).T
    xq_dev = _DEV.jax.make_array_from_single_device_arrays(
        (NCORES * D, SHARD), _DEV.sh, xq_parts)

    wsc_np = np.empty((NCORES * 128, N_TILES + D), np.float32)
    wt = W.T.astype(np.float32)
    for c in range(NCORES):
        wsc_np[c * 128:(c + 1) * 128, :N_TILES] = xsc[c]
        wsc_np[c * 128:(c + 1) * 128, N_TILES:] = wt
    wsc_dev = _DEV.put_global(wsc_np)

    # ---- edge bucketing: (core, stage, chunk, tile) with CAP slots each
    E = rows.shape[0]
    core = rows // SHARD
    tl = (rows % SHARD) // D
    stg = tl // T_STAGE
    tls = tl - stg * T_STAGE
    chunk = cols // CHUNK
    key = ((core * NSTAGES + stg) * N_CHUNKS + chunk) * T_STAGE + tls
    order = np.argsort(key.astype(np.int16), kind="stable")
    key_s = key[order]
    rows_s = rows[order]
    cols_s = cols[order]
    vals_s = vals[order].astype(ml_dtypes.bfloat16)

    nkeys = NCORES * NSTAGES * N_CHUNKS * T_STAGE
    cnt = np.bincount(key_s, minlength=nkeys)
    start = np.zeros(nkeys + 1, np.int64)
    np.cumsum(cnt, out=start[1:])
    rank = np.arange(E, dtype=np.int64) - start[key_s]
    keep = rank < CAP
    bucket_in = key_s % (N_CHUNKS * T_STAGE)
    pos = bucket_in * CAP + rank
    cs = key_s // (N_CHUNKS * T_STAGE)       # core*NSTAGES + stage

    kpos = pos[keep]
    kcs = cs[keep]
    g16 = kcs * ETOT_S + (kpos % 16) * (ETOT_S // 16) + kpos // 16
    g128 = kcs * ETOT_S + (kpos % 128) * (ETOT_S // 128) + kpos // 128

    nslots = NCORES * NSTAGES * ETOT_S
    gi_all = np.zeros(nslots, np.int16)
    dl_all = np.zeros(nslots, ml_dtypes.bfloat16)
    va_all = np.zeros(nslots, ml_dtypes.bfloat16)
    gi_all[g16] = (cols_s % CHUNK)[keep].astype(np.int16)
    dl_all[g128] = (rows_s & 127)[keep].astype(np.float32).astype(
        ml_dtypes.bfloat16)
    va_all[g128] = vals_s[keep]

    # per-stage global arrays; slots ordered (core, stage) -> stride pick
    gi3 = gi_all.reshape(NCORES, NSTAGES, ETOT_S)
    dl3 = dl_all.reshape(NCORES, NSTAGES, ETOT_S)
    va3 = va_all.reshape(NCORES, NSTAGES, ETOT_S)
    stage_inputs = []
    for s in range(NSTAGES):
        gi_s = np.ascontiguousarray(gi3[:, s]).reshape(
            NCORES * 16, ETOT_S // 16)
        dl_s = np.ascontiguousarray(dl3[:, s]).reshape(
            NCORES * 128, ETOT_S // 128)
        va_s = np.ascontiguousarray(va3[:, s]).reshape(
            NCORES * 128, ETOT_S // 128)
        stage_inputs.append((
            _DEV.put_global(gi_s),
            _DEV.put_global(dl_s),
            _DEV.put_global(va_s),
        ))

    # ---- dispatch both stages (async)
    outs = []
    for s in range(NSTAGES):
        sharded, _ = _DEV.runners[s]
        gi_d, dl_d, va_d = stage_inputs[s]
        (o,) = sharded(xq_dev, wsc_dev, gi_d, dl_d, va_d,
                       _DEV.iota_dev, _DEV.zout_dev[s])
        outs.append(o)

    # ---- fetch + dequantize into the final buffer
    try:
        out = np.empty((n, D), np.float32)
        shard_list = []
        for s in range(NSTAGES):
            for sd in outs[s].addressable_shards:
                sd.data.copy_to_host_async()
                shard_list.append((s, sd.index[0].start or 0, sd.data))
        for s, row0, data in shard_list:
            buf = np.asarray(data)            # [OROWS, OCOLS] int8
            c = row0 // OROWS
            glo = c * SHARD + s * OROWS       # global output row of buf[0]
            ghi = min(glo + OROWS, n)
            if ghi <= glo:
                continue
            nr = ghi - glo
            sc = np.ascontiguousarray(buf[:nr, D:]).view(np.float32)
            np.multiply(buf[:nr, :D].astype(np.float32), sc,
                        out=out[glo:ghi])
    except Exception:
        return _host_reference(x, W, rows, cols, vals)

    if not keep.all():  # host correction for overflowing buckets (exact)
        h = x @ W.T
        sp = ~keep
        vsp = vals_s[sp].astype(np.float32)
        np.add.at(out, rows_s[sp], vsp[:, None] * h[cols_s[sp]])

    return out


# revision 5
# speedup vs baseline: 2.3513x; 1.0819x over previous
"""GCNConv on 8 Trainium2 NeuronCores (Bass/Tile) — transfer-pipelined.

The device compute (projection + edge aggregation) takes <100ms; the axon
tunnel (~40MB/s each way) dominates, so the kernel is engineered around the
wire:

 - x is quantized host-side to int8 with per-row scales (12.8MB up instead
   of 25.7MB bf16); the dequant scale is fused into the projection's
   PSUM->SBUF copy (activation scale).
 - The output is quantized on-device to int8 with per-row scales packed as
   4 extra bytes per row (13.2MB down instead of 25.7MB), dequantized on
   the host. Measured end-to-end rel err 0.9e-2 vs the 2e-2 gate.
 - No donation: output params are persistent device-resident zero buffers,
   so no 25.7MB zero upload per call.
 - Work is split into two independent stage programs (each re-projects +
   AllGathers h, then aggregates half the destination tiles) so stage A's
   output fetch overlaps stage B's upload/exec on the full-duplex tunnel.
 - Per-(core,stage) edge payloads are packed into single int8 blobs
   (gidx int16 region | dest-local int8 region | vals bf16 region, read on
   device through SBUF AP bitcasts) and uploaded with async device_puts
   issued core-by-core the moment the host finishes each piece, so the
   wire never idles behind host prep.

Aggregation math is the baseline's exact scheme: edges bucketed by
(dest-core, stage, col-chunk, dest-tile) with fixed CAP=640 per bucket
(seed-0 max occupancy is 595; overflow is corrected exactly on the host),
gpsimd dma_gather pulls h[cols] per 512-edge batch, the vector engine
scales by vals and builds selection matrices, and the tensor engine
accumulates S^T @ M into PSUM — an exact f32 segment-sum.
"""
import sys

import numpy as np
import ml_dtypes

sys.path.insert(0, "/opt/trn_rl_repo")

import concourse.bass as bass
import concourse.bacc as bacc
import concourse.mybir as mybir
import concourse.tile as tile

F32 = mybir.dt.float32
BF16 = mybir.dt.bfloat16
I16 = mybir.dt.int16
I8 = mybir.dt.int8

N_NODES = 100000
D = 128
NCORES = 8
SHARD = 12544                  # 98 tiles of 128 rows per core
N_TILES = SHARD // D
N_PAD = SHARD * NCORES         # 100352
CHUNK = 25088                  # col-index window (int16-safe)
N_CHUNKS = N_PAD // CHUNK
CAP = 640                      # edges per (dest tile, col chunk) bucket
BCAP_BLOCKS = 4                # 512-edge gather batches
NSTAGES = 2
T_STAGE = N_TILES // NSTAGES   # 49 dest tiles per stage
ETOT_S = N_CHUNKS * T_STAGE * CAP  # padded edge stream per (core, stage)
NB_S = N_CHUNKS * T_STAGE      # buckets per (core, stage)
EB = 5 * ETOT_S                # blob bytes: gidx 2E | dl 1E | vals 2E
OCOLS = D + 4                  # int8 data + f32 scale bytes per row
OROWS = T_STAGE * D            # 6272 output rows per (core, stage)
MAGIC = 12582912.0             # 1.5 * 2**23: fast float32 round-to-int


def _plan_batches(n_tiles):
    """Batches for one chunk: buckets of CAP//128 blocks per dest tile,
    sliced into BCAP_BLOCKS-block batches. Returns [(n_edges, runs)]."""
    batches = []
    cur, cur_blocks = [], 0
    for t in range(n_tiles):
        nb = CAP // 128
        done = 0
        while done < nb:
            take = min(nb - done, BCAP_BLOCKS - cur_blocks)
            cur.append((t, take, done == 0, done + take == nb))
            cur_blocks += take
            done += take
            if cur_blocks == BCAP_BLOCKS:
                batches.append((cur_blocks * 128, cur))
                cur, cur_blocks = [], 0
    if cur_blocks:
        batches.append((cur_blocks * 128, cur))
    return batches


def _build_nc(tile_lo):
    """One stage program: project h = x @ W.T (dequantizing int8 x),
    AllGather h, aggregate dest tiles [tile_lo, tile_lo + T_STAGE)."""
    plan = _plan_batches(T_STAGE)
    nc = bacc.Bacc("TRN2", target_bir_lowering=False, debug=False,
                   num_devices=NCORES)

    xq = nc.dram_tensor("xq", [D, SHARD], I8, kind="ExternalInput")
    wsc = nc.dram_tensor("wsc", [128, N_TILES + D], F32,
                         kind="ExternalInput")
    eblob = nc.dram_tensor("eblob", [128, EB // 128], I8,
                           kind="ExternalInput")
    iota = nc.dram_tensor("iota", [128, 128], I8, kind="ExternalInput")
    outq = nc.dram_tensor("outq", [OROWS, OCOLS], I8, kind="ExternalOutput")

    h_local = nc.dram_tensor("h_local", [SHARD, D], BF16)
    h_full = nc.dram_tensor("h_full", [N_PAD, D], BF16, addr_space="Shared")

    with tile.TileContext(nc) as tc:
        with (
            tc.tile_pool(name="big", bufs=1) as big_pool,
            tc.tile_pool(name="proj", bufs=4) as proj_pool,
            tc.tile_pool(name="idx", bufs=3) as idx_pool,
            tc.tile_pool(name="gat", bufs=1) as gat_pool,
            tc.tile_pool(name="sca", bufs=2) as sca_pool,
            tc.tile_pool(name="sel", bufs=2) as sel_pool,
            tc.tile_pool(name="psum", bufs=2,
                         space=bass.MemorySpace.PSUM) as psum_pool,
            tc.tile_pool(name="fin", bufs=4) as fin_pool,
            tc.tile_pool(name="qs", bufs=4) as qs_pool,
        ):
            xq_sb = big_pool.tile([D, SHARD], I8)
            wsc_sb = big_pool.tile([128, N_TILES + D], F32)
            wt_sb = big_pool.tile([D, D], BF16)
            dl_sb = big_pool.tile([128, ETOT_S // 128], I8)
            va_sb = big_pool.tile([128, 2 * ETOT_S // 128], I8)
            iota_sb = big_pool.tile([128, 128], I8)
            acc_sb = big_pool.tile([128, T_STAGE, D], F32)

            nc.sync.dma_start(xq_sb[:], xq[:])
            nc.sync.dma_start(wsc_sb[:], wsc[:])
            nc.sync.dma_start(
                dl_sb[:],
                bass.AP(eblob, 2 * ETOT_S,
                        [[ETOT_S // 128, 128], [1, ETOT_S // 128]]),
            )
            nc.sync.dma_start(
                va_sb[:],
                bass.AP(eblob, 3 * ETOT_S,
                        [[2 * ETOT_S // 128, 128], [1, 2 * ETOT_S // 128]]),
            )
            nc.sync.dma_start(iota_sb[:], iota[:])
            nc.vector.memset(acc_sb[:], 0.0)
            nc.scalar.copy(wt_sb[:], wsc_sb[:, N_TILES:])
            va_bf = va_sb[:].bitcast(BF16)   # [128, ETOT_S // 128] bf16

            # projection: h_local = (xq @ W.T) * xscale, 128-row tiles
            for t in range(N_TILES):
                xb = proj_pool.tile([D, D], BF16)
                nc.scalar.copy(xb[:], xq_sb[:, t * D:(t + 1) * D])
                ps = psum_pool.tile([D, D], F32)
                nc.tensor.matmul(out=ps[:], lhsT=xb[:], rhs=wt_sb[:],
                                 start=True, stop=True)
                ht = proj_pool.tile([D, D], BF16)
                nc.scalar.activation(
                    ht[:], ps[:], mybir.ActivationFunctionType.Copy,
                    scale=wsc_sb[:, t:t + 1],
                )
                nc.sync.dma_start(
                    bass.AP(h_local, t * D * D, [[D, D], [1, D]]), ht[:]
                )

            nc.gpsimd.collective_compute(
                "AllGather",
                mybir.AluOpType.bypass,
                replica_groups=[list(range(NCORES))],
                ins=[h_local[:]],
                outs=[h_full[:]],
            )

            # aggregation over this stage's dest tiles
            stream_off = 0
            open_psum = {}
            for k in range(N_CHUNKS):
                chunk_base = k * CHUNK
                for kb, runs in plan:
                    nb = kb // 128
                    gq = idx_pool.tile([128, kb // 8], I8)
                    nc.sync.dma_start(
                        gq[:],
                        bass.AP(eblob, stream_off // 8,
                                [[0, 8], [ETOT_S // 8, 16], [1, kb // 8]]),
                    )
                    g = gat_pool.tile([128, nb, D], BF16)
                    nc.gpsimd.dma_gather(
                        out_ap=g[:],
                        in_ap=bass.AP(h_full, chunk_base * D,
                                      [[D, CHUNK], [1, D]]),
                        idxs_ap=gq[:].bitcast(I16),
                        num_idxs=kb,
                        num_idxs_reg=kb,
                        elem_size=D,
                    )
                    v0 = stream_off // 128
                    gs = sca_pool.tile([128, nb, D], BF16)
                    nc.vector.tensor_tensor(
                        out=gs[:],
                        in0=g[:],
                        in1=va_bf[:, v0:v0 + nb]
                        .unsqueeze(2).to_broadcast([128, nb, D]),
                        op=mybir.AluOpType.mult,
                    )
                    sel = sel_pool.tile([128, nb, D], BF16)
                    nc.vector.tensor_tensor(
                        out=sel[:],
                        in0=dl_sb[:, v0:v0 + nb]
                        .unsqueeze(2).to_broadcast([128, nb, D]),
                        in1=iota_sb[:].unsqueeze(1)
                        .to_broadcast([128, nb, D]),
                        op=mybir.AluOpType.is_equal,
                    )
                    b = 0
                    for t, take, first, last in runs:
                        if t in open_psum:
                            ps = open_psum[t]
                        else:
                            ps = psum_pool.tile([D, D], F32)
                            open_psum[t] = ps
                        for j in range(take):
                            nc.tensor.matmul(
                                out=ps[:],
                                lhsT=sel[:, b + j, :],
                                rhs=gs[:, b + j, :],
                                start=first and j == 0,
                                stop=last and j == take - 1,
                            )
                        b += take
                        if last:
                            nc.vector.tensor_tensor(
                                out=acc_sb[:, t, :],
                                in0=acc_sb[:, t, :],
                                in1=ps[:],
                                op=mybir.AluOpType.add,
                            )
                            del open_psum[t]
                    stream_off += kb
            assert not open_psum

            # int8 quantization with per-row scales, packed [q | scale]
            for t in range(T_STAGE):
                m = qs_pool.tile([128, 1], F32)
                nc.vector.tensor_reduce(
                    out=m[:], in_=acc_sb[:, t, :],
                    axis=mybir.AxisListType.X, op=mybir.AluOpType.max,
                    apply_absolute_value=True,
                )
                nc.vector.tensor_scalar_max(m[:], m[:], 1e-30)
                inv = qs_pool.tile([128, 1], F32)
                nc.vector.reciprocal(inv[:], m[:])
                inv127 = qs_pool.tile([128, 1], F32)
                nc.vector.tensor_scalar_mul(inv127[:], inv[:], 127.0)
                fin = fin_pool.tile([128, OCOLS], I8)
                nc.vector.tensor_scalar(
                    out=fin[:, 0:D], in0=acc_sb[:, t, :],
                    scalar1=inv127[:], scalar2=None,
                    op0=mybir.AluOpType.mult,
                )
                nc.scalar.mul(fin[:, D:OCOLS].bitcast(F32), m[:], 1.0 / 127.0)
                nc.sync.dma_start(
                    bass.AP(outq, t * D * OCOLS, [[OCOLS, D], [1, OCOLS]]),
                    fin[:],
                )

    nc.compile()
    return nc


_NCS = [_build_nc(s * T_STAGE) for s in range(NSTAGES)]


def _make_runner(nc):
    """Jitted no-donation executor for one stage program."""
    import jax
    from jax.sharding import Mesh, PartitionSpec
    from jax.experimental.shard_map import shard_map
    from concourse import bass2jax

    bass2jax.install_neuronx_cc_hook()
    assert nc.dbg_addr is None

    partition_name = (nc.partition_id_tensor.name
                      if nc.partition_id_tensor else None)
    in_names, out_names, out_avals = [], [], []
    for alloc in nc.m.functions[0].allocations:
        if not isinstance(alloc, mybir.MemoryLocationSet):
            continue
        name = alloc.memorylocations[0].name
        if alloc.kind == "ExternalInput":
            if name != partition_name:
                in_names.append(name)
        elif alloc.kind == "ExternalOutput":
            shape = tuple(alloc.tensor_shape)
            dtype = mybir.dt.np(alloc.dtype)
            out_names.append(name)
            out_avals.append(jax.core.ShapedArray(shape, dtype))
    n_params = len(in_names)
    all_in = in_names + out_names
    if partition_name is not None:
        all_in.append(partition_name)

    def _body(*args):
        operands = list(args)
        if partition_name is not None:
            operands.append(bass2jax.partition_id_tensor())
        outs = bass2jax._bass_exec_p.bind(
            *operands,
            out_avals=tuple(out_avals),
            in_names=tuple(all_in),
            out_names=tuple(out_names),
            lowering_input_output_aliases=(),
            sim_require_finite=True,
            sim_require_nnan=True,
            nc=nc,
        )
        return tuple(outs)

    devices = jax.devices()[:NCORES]
    mesh = Mesh(np.asarray(devices), ("core",))
    spec = PartitionSpec("core")
    sharded = jax.jit(
        shard_map(_body, mesh=mesh, in_specs=(spec,) * (n_params + 1),
                  out_specs=(spec,), check_rep=False),
        keep_unused=True,
    )
    return sharded, in_names


_IOTA8 = np.ascontiguousarray(
    np.broadcast_to(np.arange(128, dtype=np.int8), (128, 128)))

# key lookup by global row-tile id (rows >> 7): (core, stage, tile-in-stage)
_R7 = np.arange(N_PAD // D, dtype=np.int32)
_R7_CORE = _R7 // N_TILES
_R7_TL = _R7 % N_TILES
_R7_STG = _R7_TL // T_STAGE
_KEYROW = (((_R7_CORE * NSTAGES + _R7_STG) * N_CHUNKS) * T_STAGE
           + (_R7_TL % T_STAGE)).astype(np.int16)
_DLROW = (np.arange(N_PAD, dtype=np.int32) & 127).astype(np.int8)


class _Device:
    """Holds the jits and the persistent device-resident constants."""

    def __init__(self):
        import jax
        from jax.sharding import Mesh, NamedSharding, PartitionSpec

        self.jax = jax
        self.devices = jax.devices()[:NCORES]
        mesh = Mesh(np.asarray(self.devices), ("core",))
        self.sh = NamedSharding(mesh, PartitionSpec("core"))
        self.runners = [_make_runner(nc) for nc in _NCS]
        self.iota_dev = jax.device_put(
            np.concatenate([_IOTA8] * NCORES, axis=0), self.sh)
        self.zout_dev = [
            jax.device_put(
                np.zeros((NCORES * OROWS, OCOLS), np.int8), self.sh)
            for _ in range(NSTAGES)
        ]
        jax.block_until_ready(self.iota_dev)
        jax.block_until_ready(self.zout_dev)

    def put_global(self, arr):
        return self.jax.device_put(arr, self.sh)


try:
    _DEV = _Device()
except Exception:
    _DEV = None


def _warmup():
    if _DEV is None:
        return
    try:
        xq = _DEV.put_global(np.zeros((NCORES * D, SHARD), np.int8))
        wsc = _DEV.put_global(
            np.zeros((NCORES * 128, N_TILES + D), np.float32))
        eb = _DEV.put_global(np.zeros((NCORES * 128, EB // 128), np.int8))
        for s in range(NSTAGES):
            sharded, _ = _DEV.runners[s]
            out = sharded(xq, wsc, eb, _DEV.iota_dev, _DEV.zout_dev[s])
            _DEV.jax.block_until_ready(out)
    except Exception:
        pass


_warmup()


def _host_reference(x, W, rows, cols, vals):
    """Exact full-host fallback (used only if the device path fails)."""
    h = x @ W.T
    order = np.argsort(rows, kind="stable")
    rows_s = rows[order]
    msg = h[cols[order]] * vals[order][:, None]
    boundaries = np.searchsorted(rows_s, np.arange(N_NODES)).astype(np.int64)
    np.clip(boundaries, 0, max(len(rows_s) - 1, 0), out=boundaries)
    out = np.add.reduceat(msg, boundaries, axis=0)
    counts = np.bincount(rows, minlength=N_NODES)
    out[counts == 0] = 0.0
    return out.astype(np.float32)


def kernel(x, W, adj_rows, adj_cols, adj_vals):
    x = np.asarray(x, dtype=np.float32)
    W = np.asarray(W, dtype=np.float32)
    rows = np.asarray(adj_rows).astype(np.int32, copy=False)
    cols = np.asarray(adj_cols).astype(np.int32, copy=False)
    vals = np.asarray(adj_vals, dtype=np.float32)
    n = x.shape[0]

    if _DEV is None:
        return _host_reference(x, W, rows, cols, vals)

    jax = _DEV.jax

    # ---- x: per-row int8 quantization, per-core async upload pipeline
    xq_parts = []
    xsc = np.empty((NCORES, 128, N_TILES), np.float32)
    for c in range(NCORES):
        lo = c * SHARD
        hi = min(lo + SHARD, n)
        xc = x[lo:hi]
        m = np.abs(xc).max(axis=1)
        np.maximum(m, 1e-30, out=m)
        s = m * (1.0 / 127.0)
        buf = xc * (1.0 / s)[:, None]
        buf += MAGIC
        i32 = buf.view(np.int32)
        i32 -= 0x4B400000
        q8 = i32.astype(np.int8)
        qt = np.zeros((D, SHARD), np.int8)
        qt[:, :xc.shape[0]] = q8.T
        xq_parts.append(jax.device_put(qt, _DEV.devices[c]))
        s_pad = np.full(SHARD, 1.0, np.float32)
        s_pad[:xc.shape[0]] = s
        xsc[c] = s_pad.reshape(N_TILES, 128).T
    xq_dev = jax.make_array_from_single_device_arrays(
        (NCORES * D, SHARD), _DEV.sh, xq_parts)

    wsc_np = np.empty((NCORES * 128, N_TILES + D), np.float32)
    wt = W.T.astype(np.float32)
    for c in range(NCORES):
        wsc_np[c * 128:(c + 1) * 128, :N_TILES] = xsc[c]
        wsc_np[c * 128:(c + 1) * 128, N_TILES:] = wt
    wsc_dev = _DEV.put_global(wsc_np)

    # ---- edge bucketing: (core, stage, chunk, tile) with CAP slots each
    E = rows.shape[0]
    key = _KEYROW[rows >> 7] + (cols // CHUNK).astype(np.int16) * T_STAGE
    order = np.argsort(key, kind="stable")
    key_s = key[order].astype(np.int32)
    dl_s = (rows & 127).astype(np.int8)[order]
    cw_s = (cols % CHUNK).astype(np.int16)[order]
    vals_b = vals.astype(ml_dtypes.bfloat16)
    va_s = vals_b[order]

    nkeys = NCORES * NSTAGES * NB_S
    cnt = np.bincount(key_s, minlength=nkeys)
    startb = np.zeros(nkeys + 1, np.int64)
    np.cumsum(cnt, out=startb[1:])
    rank = (np.arange(E, dtype=np.int64) - startb[key_s]).astype(np.int32)
    keep = rank < CAP

    # per-(core, stage) blob assembly + async puts, stage A cores first
    blob_parts = [[None] * NCORES for _ in range(NSTAGES)]
    spill = []
    for s in range(NSTAGES):
        for c in range(NCORES):
            cs = c * NSTAGES + s
            lo = startb[cs * NB_S]
            hi = startb[(cs + 1) * NB_S]
            sl = slice(lo, hi)
            kl = keep[sl]
            if not kl.all():
                spill.append((sl, kl))
            bkt = key_s[sl] - cs * NB_S
            pos = bkt[kl] * CAP + rank[sl][kl]
            blob = np.zeros(EB, np.uint8)
            gi = blob[:2 * ETOT_S].view(np.int16)
            gi[(pos % 16) * (ETOT_S // 16) + pos // 16] = cw_s[sl][kl]
            g128 = (pos % 128) * (ETOT_S // 128) + pos // 128
            blob[2 * ETOT_S:3 * ETOT_S].view(np.int8)[g128] = dl_s[sl][kl]
            blob[3 * ETOT_S:].view(ml_dtypes.bfloat16)[g128] = va_s[sl][kl]
            blob_parts[s][c] = jax.device_put(
                blob.view(np.int8).reshape(128, EB // 128), _DEV.devices[c])

    eblob_dev = [
        jax.make_array_from_single_device_arrays(
            (NCORES * 128, EB // 128), _DEV.sh, blob_parts[s])
        for s in range(NSTAGES)
    ]

    # ---- dispatch both stages (async)
    outs = []
    for s in range(NSTAGES):
        sharded, _ = _DEV.runners[s]
        (o,) = sharded(xq_dev, wsc_dev, eblob_dev[s],
                       _DEV.iota_dev, _DEV.zout_dev[s])
        outs.append(o)

    # ---- fetch + dequantize into the final buffer
    try:
        out = np.empty((n, D), np.float32)
        shard_list = []
        for s in range(NSTAGES):
            for sd in outs[s].addressable_shards:
                sd.data.copy_to_host_async()
                shard_list.append((s, sd.index[0].start or 0, sd.data))
        for s, row0, data in shard_list:
            buf = np.asarray(data)            # [OROWS, OCOLS] int8
            c = row0 // OROWS
            glo = c * SHARD + s * OROWS       # global output row of buf[0]
            ghi = min(glo + OROWS, n)
            if ghi <= glo:
                continue
            nr = ghi - glo
            sc = np.ascontiguousarray(buf[:nr, D:]).view(np.float32)
            np.multiply(buf[:nr, :D].astype(np.float32), sc,
                        out=out[glo:ghi])
    except Exception:
        return _host_reference(x, W, rows, cols, vals)

    if spill:  # host correction for overflowing buckets (exact)
        h = x @ W.T
        rows_s = rows[order]
        cols_s = cols[order]
        for sl, kl in spill:
            sp = ~kl
            vsp = va_s[sl][sp].astype(np.float32)
            np.add.at(out, rows_s[sl][sp],
                      vsp[:, None] * h[cols_s[sl][sp]])

    return out


# revision 6
# speedup vs baseline: 2.4810x; 1.0552x over previous
"""GCNConv on 8 Trainium2 NeuronCores (Bass/Tile) — transfer-pipelined.

The device compute (projection + edge aggregation) takes <100ms; the axon
tunnel (~40MB/s each way) dominates, so the kernel is engineered around the
wire:

 - x is quantized host-side to int8 with per-row scales (12.8MB up instead
   of 25.7MB bf16); the dequant scale is fused into the projection's
   PSUM->SBUF copy (activation scale).
 - The output is quantized on-device to int8 with per-row scales packed as
   4 extra bytes per row (13.2MB down instead of 25.7MB), dequantized on
   the host. Measured end-to-end rel err 0.9e-2 vs the 2e-2 gate.
 - No donation: output params are persistent device-resident zero buffers,
   so no 25.7MB zero upload per call.
 - Work is split into four stage invocations of one program (each
   re-projects + AllGathers h, then aggregates a quarter of the
   destination tiles) so early stages' output fetches overlap later
   stages' uploads/exec on the full-duplex tunnel.
 - Per-(core,stage) edge payloads are packed into single int8 blobs
   (gidx int16 region | dest-local int8 region | vals bf16 region, read on
   device through SBUF AP bitcasts) and uploaded with async device_puts
   issued core-by-core the moment the host finishes each piece, so the
   wire never idles behind host prep.

Aggregation math is the baseline's exact scheme: edges bucketed by
(dest-core, stage-tile, col-chunk) with fixed CAP=640 per bucket (seed-0
max occupancy is 595; overflow is corrected exactly on the host), gpsimd
dma_gather pulls h[cols] per 640-edge bucket, the vector engine scales by
vals and builds selection matrices, and the tensor engine accumulates
S^T @ M into one PSUM tile per dest tile — an exact f32 segment-sum.
"""
import sys

import numpy as np
import ml_dtypes

sys.path.insert(0, "/opt/trn_rl_repo")

import concourse.bass as bass
import concourse.bacc as bacc
import concourse.mybir as mybir
import concourse.tile as tile

F32 = mybir.dt.float32
BF16 = mybir.dt.bfloat16
I16 = mybir.dt.int16
I8 = mybir.dt.int8

N_NODES = 100000
D = 128
NCORES = 8
SHARD = 12544                  # 98 tiles of 128 rows per core
N_TILES = SHARD // D
N_PAD = SHARD * NCORES         # 100352
CHUNK = 25088                  # col-index window (int16-safe)
N_CHUNKS = N_PAD // CHUNK
CAP = 640                      # edges per (dest tile, col chunk) bucket
NSTAGES = 4
T_STAGE = 25                   # tiles per stage (98 real + 2 pad)
NB_S = T_STAGE * N_CHUNKS      # buckets per (core, stage), tile-major
ETOT_S = NB_S * CAP            # padded edge slots per (core, stage)
EB = 5 * ETOT_S                # blob bytes: gidx 2E | dl 1E | vals 2E
OCOLS = D + 4                  # int8 data + f32 scale bytes per row
OROWS = T_STAGE * D            # 3200 output rows per (core, stage)
MAGIC = 12582912.0             # 1.5 * 2**23: fast float32 round-to-int


def _build_nc():
    """Stage program: project h = x @ W.T (dequantizing int8 x), AllGather
    h, aggregate T_STAGE dest tiles from the edge blob (tile-major, one
    640-edge gather per (tile, chunk) bucket, one PSUM tile per dest
    tile), emit int8 output rows with packed per-row f32 scales."""
    nc = bacc.Bacc("TRN2", target_bir_lowering=False, debug=False,
                   num_devices=NCORES)

    xq = nc.dram_tensor("xq", [D, SHARD], I8, kind="ExternalInput")
    wsc = nc.dram_tensor("wsc", [128, N_TILES + D], F32,
                         kind="ExternalInput")
    eblob = nc.dram_tensor("eblob", [128, EB // 128], I8,
                           kind="ExternalInput")
    iota = nc.dram_tensor("iota", [128, 128], I8, kind="ExternalInput")
    outq = nc.dram_tensor("outq", [OROWS, OCOLS], I8, kind="ExternalOutput")

    h_local = nc.dram_tensor("h_local", [SHARD, D], BF16)
    h_full = nc.dram_tensor("h_full", [N_PAD, D], BF16, addr_space="Shared")

    NBLK = CAP // 128          # 5 gather blocks per bucket

    with tile.TileContext(nc) as tc:
        with (
            tc.tile_pool(name="big", bufs=1) as big_pool,
            tc.tile_pool(name="proj", bufs=4) as proj_pool,
            tc.tile_pool(name="idx", bufs=3) as idx_pool,
            tc.tile_pool(name="gat", bufs=2) as gat_pool,
            tc.tile_pool(name="sca", bufs=2) as sca_pool,
            tc.tile_pool(name="sel", bufs=2) as sel_pool,
            tc.tile_pool(name="psum", bufs=4,
                         space=bass.MemorySpace.PSUM) as psum_pool,
            tc.tile_pool(name="fin", bufs=4) as fin_pool,
            tc.tile_pool(name="qs", bufs=4) as qs_pool,
        ):
            xq_sb = big_pool.tile([D, SHARD], I8)
            wsc_sb = big_pool.tile([128, N_TILES + D], F32)
            wt_sb = big_pool.tile([D, D], BF16)
            dl_sb = big_pool.tile([128, ETOT_S // 128], I8)
            va_sb = big_pool.tile([128, 2 * ETOT_S // 128], I8)
            iota_sb = big_pool.tile([128, 128], I8)

            nc.sync.dma_start(xq_sb[:], xq[:])
            nc.sync.dma_start(wsc_sb[:], wsc[:])
            nc.sync.dma_start(
                dl_sb[:],
                bass.AP(eblob, 2 * ETOT_S,
                        [[ETOT_S // 128, 128], [1, ETOT_S // 128]]),
            )
            nc.sync.dma_start(
                va_sb[:],
                bass.AP(eblob, 3 * ETOT_S,
                        [[2 * ETOT_S // 128, 128], [1, 2 * ETOT_S // 128]]),
            )
            nc.sync.dma_start(iota_sb[:], iota[:])
            nc.scalar.copy(wt_sb[:], wsc_sb[:, N_TILES:])
            va_bf = va_sb[:].bitcast(BF16)   # [128, ETOT_S // 128] bf16

            # projection: h_local = (xq @ W.T) * xscale, 128-row tiles
            for t in range(N_TILES):
                xb = proj_pool.tile([D, D], BF16)
                nc.scalar.copy(xb[:], xq_sb[:, t * D:(t + 1) * D])
                ps = psum_pool.tile([D, D], F32)
                nc.tensor.matmul(out=ps[:], lhsT=xb[:], rhs=wt_sb[:],
                                 start=True, stop=True)
                ht = proj_pool.tile([D, D], BF16)
                nc.scalar.activation(
                    ht[:], ps[:], mybir.ActivationFunctionType.Copy,
                    scale=wsc_sb[:, t:t + 1],
                )
                nc.sync.dma_start(
                    bass.AP(h_local, t * D * D, [[D, D], [1, D]]), ht[:]
                )

            nc.gpsimd.collective_compute(
                "AllGather",
                mybir.AluOpType.bypass,
                replica_groups=[list(range(NCORES))],
                ins=[h_local[:]],
                outs=[h_full[:]],
            )

            # aggregation: tile-major, one 640-edge bucket per (tile, chunk)
            for t in range(T_STAGE):
                ps = psum_pool.tile([D, D], F32)
                for k in range(N_CHUNKS):
                    stream_off = (t * N_CHUNKS + k) * CAP
                    v0 = stream_off // 128
                    gq = idx_pool.tile([128, CAP // 8], I8)
                    nc.sync.dma_start(
                        gq[:],
                        bass.AP(eblob, stream_off // 8,
                                [[0, 8], [ETOT_S // 8, 16], [1, CAP // 8]]),
                    )
                    g = gat_pool.tile([128, NBLK, D], BF16)
                    nc.gpsimd.dma_gather(
                        out_ap=g[:],
                        in_ap=bass.AP(h_full, k * CHUNK * D,
                                      [[D, CHUNK], [1, D]]),
                        idxs_ap=gq[:].bitcast(I16),
                        num_idxs=CAP,
                        num_idxs_reg=CAP,
                        elem_size=D,
                    )
                    gs = sca_pool.tile([128, NBLK, D], BF16)
                    nc.vector.tensor_tensor(
                        out=gs[:],
                        in0=g[:],
                        in1=va_bf[:, v0:v0 + NBLK]
                        .unsqueeze(2).to_broadcast([128, NBLK, D]),
                        op=mybir.AluOpType.mult,
                    )
                    sel = sel_pool.tile([128, NBLK, D], BF16)
                    nc.vector.tensor_tensor(
                        out=sel[:],
                        in0=dl_sb[:, v0:v0 + NBLK]
                        .unsqueeze(2).to_broadcast([128, NBLK, D]),
                        in1=iota_sb[:].unsqueeze(1)
                        .to_broadcast([128, NBLK, D]),
                        op=mybir.AluOpType.is_equal,
                    )
                    for j in range(NBLK):
                        nc.tensor.matmul(
                            out=ps[:],
                            lhsT=sel[:, j, :],
                            rhs=gs[:, j, :],
                            start=(k == 0 and j == 0),
                            stop=(k == N_CHUNKS - 1 and j == NBLK - 1),
                        )

                # int8 quantization with per-row scale, packed [q | scale]
                m = qs_pool.tile([128, 1], F32)
                nc.vector.tensor_reduce(
                    out=m[:], in_=ps[:],
                    axis=mybir.AxisListType.X, op=mybir.AluOpType.max,
                    apply_absolute_value=True,
                )
                nc.vector.tensor_scalar_max(m[:], m[:], 1e-30)
                inv = qs_pool.tile([128, 1], F32)
                nc.vector.reciprocal(inv[:], m[:])
                inv127 = qs_pool.tile([128, 1], F32)
                nc.vector.tensor_scalar_mul(inv127[:], inv[:], 127.0)
                fin = fin_pool.tile([128, OCOLS], I8)
                nc.vector.tensor_scalar(
                    out=fin[:, 0:D], in0=ps[:],
                    scalar1=inv127[:], scalar2=None,
                    op0=mybir.AluOpType.mult,
                )
                nc.scalar.mul(fin[:, D:OCOLS].bitcast(F32), m[:], 1.0 / 127.0)
                nc.sync.dma_start(
                    bass.AP(outq, t * D * OCOLS, [[OCOLS, D], [1, OCOLS]]),
                    fin[:],
                )

    nc.compile()
    return nc


_NC = _build_nc()


def _make_runner(nc):
    """Jitted no-donation executor for the stage program."""
    import jax
    from jax.sharding import Mesh, PartitionSpec
    from jax.experimental.shard_map import shard_map
    from concourse import bass2jax

    bass2jax.install_neuronx_cc_hook()
    assert nc.dbg_addr is None

    partition_name = (nc.partition_id_tensor.name
                      if nc.partition_id_tensor else None)
    in_names, out_names, out_avals = [], [], []
    for alloc in nc.m.functions[0].allocations:
        if not isinstance(alloc, mybir.MemoryLocationSet):
            continue
        name = alloc.memorylocations[0].name
        if alloc.kind == "ExternalInput":
            if name != partition_name:
                in_names.append(name)
        elif alloc.kind == "ExternalOutput":
            shape = tuple(alloc.tensor_shape)
            dtype = mybir.dt.np(alloc.dtype)
            out_names.append(name)
            out_avals.append(jax.core.ShapedArray(shape, dtype))
    n_params = len(in_names)
    all_in = in_names + out_names
    if partition_name is not None:
        all_in.append(partition_name)

    def _body(*args):
        operands = list(args)
        if partition_name is not None:
            operands.append(bass2jax.partition_id_tensor())
        outs = bass2jax._bass_exec_p.bind(
            *operands,
            out_avals=tuple(out_avals),
            in_names=tuple(all_in),
            out_names=tuple(out_names),
            lowering_input_output_aliases=(),
            sim_require_finite=True,
            sim_require_nnan=True,
            nc=nc,
        )
        return tuple(outs)

    devices = jax.devices()[:NCORES]
    mesh = Mesh(np.asarray(devices), ("core",))
    spec = PartitionSpec("core")
    sharded = jax.jit(
        shard_map(_body, mesh=mesh, in_specs=(spec,) * (n_params + 1),
                  out_specs=(spec,), check_rep=False),
        keep_unused=True,
    )
    return sharded


_IOTA8 = np.ascontiguousarray(
    np.broadcast_to(np.arange(128, dtype=np.int8), (128, 128)))

# per-row-tile key LUT: rows >> 7 -> core*NSTAGES*NB_S + stage*NB_S + tls*4
_R7 = np.arange(N_PAD // D, dtype=np.int32)
_R7_TL = _R7 % N_TILES
_KEYROW = (_R7 // N_TILES * (NSTAGES * NB_S)
           + (_R7_TL // T_STAGE) * NB_S
           + (_R7_TL % T_STAGE) * N_CHUNKS).astype(np.int16)


class _Device:
    """Holds the jit and the persistent device-resident constants."""

    def __init__(self):
        import jax
        from jax.sharding import Mesh, NamedSharding, PartitionSpec

        self.jax = jax
        self.devices = jax.devices()[:NCORES]
        mesh = Mesh(np.asarray(self.devices), ("core",))
        self.sh = NamedSharding(mesh, PartitionSpec("core"))
        self.runner = _make_runner(_NC)
        self.iota_dev = jax.device_put(
            np.concatenate([_IOTA8] * NCORES, axis=0), self.sh)
        self.zout_dev = jax.device_put(
            np.zeros((NCORES * OROWS, OCOLS), np.int8), self.sh)
        jax.block_until_ready(self.iota_dev)
        jax.block_until_ready(self.zout_dev)

    def put_global(self, arr):
        return self.jax.device_put(arr, self.sh)


try:
    _DEV = _Device()
except Exception:
    _DEV = None


def _warmup():
    if _DEV is None:
        return
    try:
        xq = _DEV.put_global(np.zeros((NCORES * D, SHARD), np.int8))
        wsc = _DEV.put_global(
            np.zeros((NCORES * 128, N_TILES + D), np.float32))
        eb = _DEV.put_global(np.zeros((NCORES * 128, EB // 128), np.int8))
        out = _DEV.runner(xq, wsc, eb, _DEV.iota_dev, _DEV.zout_dev)
        _DEV.jax.block_until_ready(out)
    except Exception:
        pass


_warmup()


def _host_reference(x, W, rows, cols, vals):
    """Exact full-host fallback (used only if the device path fails)."""
    h = x @ W.T
    order = np.argsort(rows, kind="stable")
    rows_s = rows[order]
    msg = h[cols[order]] * vals[order][:, None]
    boundaries = np.searchsorted(rows_s, np.arange(N_NODES)).astype(np.int64)
    np.clip(boundaries, 0, max(len(rows_s) - 1, 0), out=boundaries)
    out = np.add.reduceat(msg, boundaries, axis=0)
    counts = np.bincount(rows, minlength=N_NODES)
    out[counts == 0] = 0.0
    return out.astype(np.float32)


def kernel(x, W, adj_rows, adj_cols, adj_vals):
    x = np.asarray(x, dtype=np.float32)
    W = np.asarray(W, dtype=np.float32)
    rows = np.asarray(adj_rows).astype(np.int32, copy=False)
    cols = np.asarray(adj_cols).astype(np.int32, copy=False)
    vals = np.asarray(adj_vals, dtype=np.float32)
    n = x.shape[0]

    if _DEV is None:
        return _host_reference(x, W, rows, cols, vals)

    jax = _DEV.jax

    # ---- x: per-row int8 quantization, per-core async upload pipeline
    xq_parts = []
    xsc = np.empty((NCORES, 128, N_TILES), np.float32)
    for c in range(NCORES):
        lo = c * SHARD
        hi = min(lo + SHARD, n)
        xc = x[lo:hi]
        m = np.abs(xc).max(axis=1)
        np.maximum(m, 1e-30, out=m)
        s = m * (1.0 / 127.0)
        buf = xc * (1.0 / s)[:, None]
        buf += MAGIC
        i32 = buf.view(np.int32)
        i32 -= 0x4B400000
        q8 = i32.astype(np.int8)
        qt = np.zeros((D, SHARD), np.int8)
        qt[:, :xc.shape[0]] = q8.T
        xq_parts.append(jax.device_put(qt, _DEV.devices[c]))
        s_pad = np.full(SHARD, 1.0, np.float32)
        s_pad[:xc.shape[0]] = s
        xsc[c] = s_pad.reshape(N_TILES, 128).T
    xq_dev = jax.make_array_from_single_device_arrays(
        (NCORES * D, SHARD), _DEV.sh, xq_parts)

    wsc_np = np.empty((NCORES * 128, N_TILES + D), np.float32)
    wt = W.T.astype(np.float32)
    for c in range(NCORES):
        wsc_np[c * 128:(c + 1) * 128, :N_TILES] = xsc[c]
        wsc_np[c * 128:(c + 1) * 128, N_TILES:] = wt
    wsc_dev = _DEV.put_global(wsc_np)

    # ---- edge bucketing: (core, stage, tile, chunk) with CAP slots each
    E = rows.shape[0]
    key = _KEYROW[rows >> 7] + (cols // CHUNK).astype(np.int16)
    order = np.argsort(key, kind="stable")
    key_s = key[order].astype(np.int32)
    dl_s = (rows & 127).astype(np.int8)[order]
    cw_s = (cols % CHUNK).astype(np.int16)[order]
    va_s = vals.astype(ml_dtypes.bfloat16)[order]

    nkeys = NCORES * NSTAGES * NB_S
    cnt = np.bincount(key_s, minlength=nkeys)
    startb = np.zeros(nkeys + 1, np.int64)
    np.cumsum(cnt, out=startb[1:])
    rank = (np.arange(E, dtype=np.int64) - startb[key_s]).astype(np.int32)
    keep = rank < CAP

    # per-(core, stage) blob assembly + async puts, stage-major issue order
    blob_parts = [[None] * NCORES for _ in range(NSTAGES)]
    spill = []
    for s in range(NSTAGES):
        for c in range(NCORES):
            cs = c * NSTAGES + s
            lo = startb[cs * NB_S]
            hi = startb[(cs + 1) * NB_S]
            sl = slice(lo, hi)
            kl = keep[sl]
            if not kl.all():
                spill.append((sl, kl))
            bkt = key_s[sl] - cs * NB_S
            pos = bkt[kl] * CAP + rank[sl][kl]
            blob = np.zeros(EB, np.uint8)
            gi = blob[:2 * ETOT_S].view(np.int16)
            gi[(pos % 16) * (ETOT_S // 16) + pos // 16] = cw_s[sl][kl]
            g128 = (pos % 128) * (ETOT_S // 128) + pos // 128
            blob[2 * ETOT_S:3 * ETOT_S].view(np.int8)[g128] = dl_s[sl][kl]
            blob[3 * ETOT_S:].view(ml_dtypes.bfloat16)[g128] = va_s[sl][kl]
            blob_parts[s][c] = jax.device_put(
                blob.view(np.int8).reshape(128, EB // 128), _DEV.devices[c])

    # ---- dispatch all stages (async; one jit, data-dependent start)
    outs = []
    for s in range(NSTAGES):
        eblob_dev = jax.make_array_from_single_device_arrays(
            (NCORES * 128, EB // 128), _DEV.sh, blob_parts[s])
        (o,) = _DEV.runner(xq_dev, wsc_dev, eblob_dev,
                           _DEV.iota_dev, _DEV.zout_dev)
        outs.append(o)

    # ---- fetch + dequantize into the final buffer
    try:
        out = np.empty((n, D), np.float32)
        shard_list = []
        for s in range(NSTAGES):
            for sd in outs[s].addressable_shards:
                sd.data.copy_to_host_async()
                shard_list.append((s, sd.index[0].start or 0, sd.data))
        for s, row0, data in shard_list:
            buf = np.asarray(data)            # [OROWS, OCOLS] int8
            c = row0 // OROWS
            glo = c * SHARD + s * OROWS       # global output row of buf[0]
            ghi = min(glo + OROWS, (c + 1) * SHARD, n)
            if ghi <= glo:
                continue
            nr = ghi - glo
            sc = np.ascontiguousarray(buf[:nr, D:]).view(np.float32)
            np.multiply(buf[:nr, :D], sc, out=out[glo:ghi])
    except Exception:
        return _host_reference(x, W, rows, cols, vals)

    if spill:  # host correction for overflowing buckets (exact)
        h = x @ W.T
        rows_s = rows[order]
        cols_s = cols[order]
        for sl, kl in spill:
            sp = ~kl
            vsp = va_s[sl][sp].astype(np.float32)
            np.add.at(out, rows_s[sl][sp],
                      vsp[:, None] * h[cols_s[sl][sp]])

    return out
